# revision 1
# baseline (speedup 1.0000x reference)
"""Trainium2 Bass kernel for nn_BSLoss (Black-Scholes PINN loss on a 4096x4096 grid).

Strategy (8 NeuronCores, SPMD, S-sharded):
  - Each core takes 512 grid rows (+1-row halos, host-sliced) x all 4096 t-columns,
    processed as 4 x 128-row tiles (step 126, interior outputs only) + a 10-row strip.
  - PDE residual r/c = tri_S(V)/c + (V[:,j+1] - V[:,j-1]). The S-stencil is a
    [128x128] tridiagonal float32r (tf32) stationary matmul per 512-col chunk
    (PE contracts over partitions, so cross-partition shifts come for free). The
    t-stencil is two +/-identity float32r matmuls against column-shifted moving
    APs accumulating into the same PSUM bank (strip tiles instead use a DVE
    tensor-sub + scalar_tensor_tensor into PSUM).
  - Chunks are processed as pairs in 2-bank [128, 1024] PSUM group tiles (4 in
    flight); each group is squared+reduced by one ScalarEngine
    activation(Square, accum_out) - or DVE copy+STT for two tiles, to balance
    engines - into per-row stats[128, 20].
  - Host applies row masks (x C_T^2, the folded-out time-step scale) to the tiny
    stats, sums in float64, and computes the O(N) boundary losses (rows 0/4095,
    col 4095) directly.

Grid boundary columns (t=0, t=4095) are excluded by construction: chunks cover
columns 1..4094 only.
"""
import os
import sys

if "/opt/trn_rl_repo" not in sys.path:
    sys.path.insert(0, "/opt/trn_rl_repo")

import numpy as np

import concourse.mybir as mybir
import concourse.tile as tile
from concourse import bacc
from concourse.bass_utils import run_bass_kernel_spmd

# ---- problem constants (match the reference) ----
N_S, N_T = 4096, 4096
R, SIGMA, K, T_MAT, SMAX = 0.05, 0.2, 100.0, 1.0, 300.0
B_STR, ALPHA = K / SMAX, 0.5
L_PDE, L_BC, L_TC = 1.0, 10.0, 10.0
HUBER_DELTA = 0.01
SOFTPLUS_BETA = 50.0

N_CORES = 8
ROWS_PER_CORE = N_S // N_CORES          # 512
IN_ROWS = ROWS_PER_CORE + 2             # 514 (with halos)
P = 128
TILE_STARTS = [0, 126, 252, 378]        # full tiles; outputs local rows 1..504
STRIP_START = 504                       # strip tile rows 504..513 -> outputs 505..512
STRIP_K = 10
N_TILES = 5
C_T = (N_T - 1) / 2.0 / T_MAT           # 2047.5

# column halves: half 0 = global cols [0, 2056); half 1 = [2040, 4096)
HALF_OFF = [0, 2040]
HALF_W = 2056
# chunk starts in GLOBAL columns; groups = chunk pairs (2 PSUM banks each)
# indexed by (h, s): h = column half (for the half-tile SBUF layout), s = subgroup
CHUNKS_G = {
    (0, 0): [(1, 512), (513, 512)],
    (0, 1): [(1025, 512), (1537, 512)],
    (1, 0): [(2049, 512), (2561, 512)],
    (1, 1): [(3073, 512), (3585, 510)],
}
GROUP_W = {(0, 0): 1024, (0, 1): 1024, (1, 0): 1024, (1, 1): 1022}
N_GROUPS = N_TILES * 4
# groups whose t-stencil runs on DVE (TT sub + STT accumulate into PSUM)
DVE_SHIFT = {(4, 0), (4, 1)}
DVE_SQACC = {(1, 1), (2, 1)}

F32 = mybir.dt.float32
F32R = mybir.dt.float32r


def _solve_cubic(Q: float) -> float:
    c = -Q
    for _ in range(5):
        f = c ** 3 / 6.0 + c + Q
        df = 0.5 * c * c + 1.0
        c = c - f / df
    return c


C1 = _solve_cubic((B_STR - 0.0) / ALPHA)
C2 = _solve_cubic((B_STR - 1.0) / ALPHA)


def _tf32(x: np.ndarray) -> np.ndarray:
    """Round float32 to tfloat32 (10-bit mantissa, round-to-nearest)."""
    u = np.ascontiguousarray(x, dtype=np.float32).view(np.uint32).astype(np.uint64)
    u = (u + np.uint64(0x1000)) & np.uint64(0xFFFFE000)
    return u.astype(np.uint32).view(np.float32)


def _stencil_coeffs(S: np.ndarray):
    """Per-row stencil coefficients / C_T (c folded out; re-applied via host mask)."""
    S = S.astype(np.float64)
    dS = 1.0 / (N_S - 1)
    L = C2 * S + C1 * (1.0 - S)
    dL = C2 - C1
    S_u = ALPHA * dL * (0.5 * L ** 2 + 1.0)
    S_uu = ALPHA * dL ** 2 * L
    e = 0.5 * SIGMA ** 2 * S ** 2
    f = R * S
    a_uu = e / S_u ** 2
    a_u = f / S_u - e * S_uu / S_u ** 3
    hi = a_uu / dS ** 2 + a_u / (2 * dS)
    lo = a_uu / dS ** 2 - a_u / (2 * dS)
    mid = -2.0 * a_uu / dS ** 2 - R
    return lo / C_T, mid / C_T, hi / C_T


_PROGRAM = None


def _patch_tail(tc_cls):
    """Cheaper kernel tail: drain + single barrier, no per-sem HW clears.
    Semaphore bookkeeping (free/poison) is kept so scheduling stays valid."""
    import concourse.tile as _tile
    from concourse.vector_clock import ScopedClock as _SC

    def _drain_and_barrier(self, tick_clock, wait_clock):
        drain_inst = self.nc.sync.drain()
        wait_clock.add_sem_waits(drain_inst.ins, _SC({None: tick_clock.global_clock}))
        self.nc.all_engine_barrier()
        popped = self.nc._tile_sem_poison_stack.pop()
        assert popped is self._sem_poison
        sems = list(self.sems.allocated().values())
        sem_nums = [s.num if hasattr(s, "num") else s for s in sems]
        self.nc._state.prepend_free_semaphores(sem_nums)
        for poison_set in self.nc._tile_sem_poison_stack:
            poison_set.update(sem_nums)

    tc_cls._drain_and_barrier = _drain_and_barrier


def _build_program():
    if os.environ.get("BSLOSS_FAST_TAIL", "1") == "1":
        _patch_tail(tile.TileContext)
    nc = bacc.Bacc("TRN2", target_bir_lowering=False)

    v_in = nc.dram_tensor("v_in", [IN_ROWS, N_T], F32R, kind="ExternalInput")
    # 5 tridiag blocks + (+I, -I) identity pair, all f32r
    w_in = nc.dram_tensor("w_in", [P, (N_TILES + 2) * P], F32R, kind="ExternalInput")
    stats_out = nc.dram_tensor("stats_out", [P, N_GROUPS], F32, kind="ExternalOutput")

    # DMA the tiny strip first (arrives early); emit its compute last so the
    # scheduler gives casts/matmuls of the big tiles priority on DVE/PE
    DMA_ORDER = [4, 0, 1, 2, 3]
    COMPUTE_ORDER = [4, 0, 1, 2, 3]

    with tile.TileContext(nc) as tc:
        with (
            tc.tile_pool(name="vpool", bufs=1) as vpool,
            tc.tile_pool(name="wpool", bufs=1) as wpool,
            tc.tile_pool(name="scratch", bufs=2) as spool,
            tc.tile_pool(name="psum", bufs=4, space="PSUM") as psum_pool,
        ):
            wall = wpool.tile([P, (N_TILES + 2) * P], F32R)
            nc.sync.dma_start(wall[:], w_in[:])
            stats = wpool.tile([P, N_GROUPS], F32)

            # v tiles in column halves, DMA'd strip-first
            vh = {}
            for t in DMA_ORDER:
                kdim = P if t < 4 else STRIP_K
                r0 = TILE_STARTS[t] if t < 4 else STRIP_START
                for h in (0, 1):
                    vt = vpool.tile([kdim, HALF_W], F32R, tag=f"v{t}{h}")
                    nc.sync.dma_start(
                        vt[:], v_in[r0:r0 + kdim, HALF_OFF[h]:HALF_OFF[h] + HALF_W])
                    vh[(t, h)] = vt

            def vslice(t, h, c0, c1):
                """AP for global columns [c0, c1) within half (t, h)."""
                kdim = P if t < 4 else STRIP_K
                return vh[(t, h)][0:kdim, c0 - HALF_OFF[h]:c1 - HALF_OFF[h]]

            for t in COMPUTE_ORDER:
                kdim = P if t < 4 else STRIP_K
                tri = wall[0:kdim, t * P:(t + 1) * P]
                for h in (0, 1):
                    for s in (0, 1):
                        u = (t * 2 + h) * 2 + s
                        gw = GROUP_W[(h, s)]
                        chunks = CHUNKS_G[(h, s)]
                        ps = psum_pool.tile([P, 1024], F32, tag="ps")
                        dve_shift = (t, h) in DVE_SHIFT
                        for ci, (ga, cw) in enumerate(chunks):
                            nc.tensor.matmul(ps[:, 512 * ci:512 * ci + cw],
                                             lhsT=tri,
                                             rhs=vslice(t, h, ga, ga + cw),
                                             start=True, stop=dve_shift)
                        if dve_shift:
                            g0 = chunks[0][0]
                            wsc = spool.tile([P, 1024], F32, tag="w")
                            nc.vector.tensor_tensor(
                                out=wsc[0:kdim, 0:gw],
                                in0=vslice(t, h, g0 + 1, g0 + 1 + gw).bitcast(F32),
                                in1=vslice(t, h, g0 - 1, g0 - 1 + gw).bitcast(F32),
                                op=mybir.AluOpType.subtract)
                            nc.vector.scalar_tensor_tensor(
                                out=ps[0:kdim, 0:gw], in0=wsc[0:kdim, 0:gw],
                                scalar=1.0, in1=ps[0:kdim, 0:gw],
                                op0=mybir.AluOpType.mult, op1=mybir.AluOpType.add)
                        else:
                            for sweep, sh in enumerate((1, -1)):
                                ident = wall[0:kdim, (N_TILES + sweep) * P:
                                             (N_TILES + sweep + 1) * P]
                                for ci, (ga, cw) in enumerate(chunks):
                                    nc.tensor.matmul(
                                        ps[:, 512 * ci:512 * ci + cw], lhsT=ident,
                                        rhs=vslice(t, h, ga + sh, ga + sh + cw),
                                        start=False, stop=(sweep == 1))
                        sq = spool.tile([P, 1024], F32, tag="sq")
                        if (t, h) in DVE_SQACC:
                            rc = spool.tile([P, 1024], F32, tag="rc")
                            nc.vector.tensor_copy(rc[:, 0:gw], ps[:, 0:gw])
                            nc.vector.scalar_tensor_tensor(
                                out=sq[:, 0:gw], in0=rc[:, 0:gw], scalar=1.0,
                                in1=rc[:, 0:gw], op0=mybir.AluOpType.mult,
                                op1=mybir.AluOpType.mult,
                                accum_out=stats[:, u:u + 1])
                        else:
                            nc.scalar.activation(sq[:, 0:gw], ps[:, 0:gw],
                                                 mybir.ActivationFunctionType.Square,
                                                 accum_out=stats[:, u:u + 1])

            for t in COMPUTE_ORDER:
                nc.sync.dma_start(stats_out[:, t * 4:(t + 1) * 4],
                                  stats[:, t * 4:(t + 1) * 4])

    nc.compile()
    return nc


def _host_inputs_and_masks(V: np.ndarray, S: np.ndarray):
    lo, mid, hi = _stencil_coeffs(S)
    c2 = float(C_T) ** 2

    in_maps = []
    masks = []

    for c in range(N_CORES):
        rows = np.clip(np.arange(512 * c - 1, 512 * c + 513), 0, N_S - 1)
        v_shard = _tf32(V[rows, :])

        wtri = np.zeros((P, (N_TILES + 2) * P), np.float64)
        wtri[:, N_TILES * P:(N_TILES + 1) * P] = np.eye(P)
        wtri[:, (N_TILES + 1) * P:(N_TILES + 2) * P] = -np.eye(P)
        mask = np.zeros((P, N_GROUPS), np.float32)
        for t in range(N_TILES):
            if t < 4:
                t0, m_lo, m_hi = TILE_STARTS[t], 1, 126
            else:
                t0, m_lo, m_hi = STRIP_START, 1, 8
            for m in range(m_lo, m_hi + 1):
                L = t0 + m
                g = 512 * c - 1 + L
                if not (1 <= g <= N_S - 2):
                    continue
                wtri[m - 1, t * P + m] = lo[g]
                wtri[m, t * P + m] = mid[g]
                wtri[m + 1, t * P + m] = hi[g]
                mask[m, t * 4:t * 4 + 4] = c2
        in_maps.append({"v_in": v_shard, "w_in": _tf32(wtri.astype(np.float32))})
        masks.append(mask)
    return in_maps, masks


_LAST_RESULTS = None  # stashed BassKernelResults (for the test harness)


def kernel(V_norm: np.ndarray, S_grid: np.ndarray, t_grid: np.ndarray):
    global _PROGRAM, _LAST_RESULTS

    V = np.asarray(V_norm, dtype=np.float32).reshape(N_S, N_T)
    S = np.asarray(S_grid, dtype=np.float32).reshape(N_S)
    t = np.asarray(t_grid, dtype=np.float32).reshape(N_T)

    if _PROGRAM is None:
        _PROGRAM = _build_program()
    nc = _PROGRAM

    in_maps, masks = _host_inputs_and_masks(V, S)
    trace = bool(os.environ.get("BSLOSS_TRACE"))
    res = run_bass_kernel_spmd(nc, in_maps, core_ids=list(range(N_CORES)),
                               trace=trace)
    _LAST_RESULTS = res

    pde_sum = 0.0
    for c in range(N_CORES):
        stats = res.results[c]["stats_out"].astype(np.float64)
        pde_sum += float((masks[c].astype(np.float64) * stats).sum())
    n_int = (N_S - 2) * (N_T - 2)
    pde_loss = pde_sum / n_int

    # ---- boundary losses on host (tiny O(N) edge terms), float64 ----
    V64 = V.astype(np.float64)
    S64 = S.astype(np.float64)
    t64 = t.astype(np.float64)

    loss_S0 = float((V64[0, :] ** 2).sum() / N_T)

    tau = 1.0 - t64
    V_ff = 1.0 - K * np.exp(-R * tau) / SMAX
    loss_Smax = float(((V64[N_S - 1, :] - V_ff) ** 2).sum() / N_T)

    x = SOFTPLUS_BETA * (S64 - K / SMAX)
    payoff = (np.maximum(x, 0.0) + np.log1p(np.exp(-np.abs(x)))) / SOFTPLUS_BETA
    diff_T = V64[:, N_T - 1] - payoff
    abs_d = np.abs(diff_T)
    huber = np.where(abs_d < HUBER_DELTA, 0.5 * diff_T ** 2,
                     HUBER_DELTA * (abs_d - 0.5 * HUBER_DELTA))
    loss_T = float(huber.sum() / N_S)

    total = L_PDE * pde_loss + L_BC * loss_Smax + L_TC * loss_T
    return (np.float32(total), np.float32(pde_loss), np.float32(loss_S0),
            np.float32(loss_Smax), np.float32(loss_T))



# revision 7
# speedup vs baseline: 1.2363x; 1.2363x over previous
"""Trainium2 Bass kernel for nn_BSLoss (Black-Scholes PINN loss on a 4096x4096 grid).

Strategy V2 (8 NeuronCores, SPMD, S-sharded, fp16 on device):
  - Host converts V to fp16 (halves DMA: ~4.4 MB/core). Loss is a mean of
    ~16.7M squared residuals, so V-rounding noise enters the sum as
    E[eps^2]/E[r^2] ~ 2^-22 -- far below the 2e-2 gate.
  - Each core takes 512 grid rows (+1-row halos, host-sliced) x all 4096
    t-cols: 4 x [128, 4096] tiles (outputs rows 1..126 each) plus a folded
    strip: the last 10 rows are stacked as 4 column-chunks x 10 rows =
    [40, ~1026], so the strip costs 1/4 sweep instead of a full one.
  - Per group of 1024 cols: residual r = tri_S(V) + (V[:,t+1] - V[:,t-1]).
    DVE computes the t-diff D in fp16 (2x mode); the PE runs the [128,128]
    tridiagonal fp16 matmul (S-stencil via partition contraction) and then
    accumulates D into the same PSUM bank via an identity matmul -- no
    second/third full-price engine pass over the data.
  - Sum of squares: ScalarE activation(Square, accum_out) from PSUM for most
    groups; a few groups go through DVE bn_stats (count/mean/M2 per 512-col
    chunk) to balance ACT vs DVE; host reconstructs sum(x^2) = M2 + n*mean^2.
  - Host applies per-row masks (x C_T^2, the folded-out time-step scale) to
    the tiny stats and computes the O(N) boundary losses in float64.

Grid boundary columns (t=0, t=4095) are excluded by construction: groups
cover columns 1..4094 only. Boundary rows are excluded by zero stencil
coefficients + host masks.
"""
import os
import sys

if "/opt/trn_rl_repo" not in sys.path:
    sys.path.insert(0, "/opt/trn_rl_repo")

import numpy as np

import concourse.mybir as mybir
import concourse.tile as tile
from concourse import bacc
from concourse.bass_utils import run_bass_kernel_spmd

# ---- problem constants (match the reference) ----
N_S, N_T = 4096, 4096
R, SIGMA, K, T_MAT, SMAX = 0.05, 0.2, 100.0, 1.0, 300.0
B_STR, ALPHA = K / SMAX, 0.5
L_PDE, L_BC, L_TC = 1.0, 10.0, 10.0
HUBER_DELTA = 0.01
SOFTPLUS_BETA = 50.0

N_CORES = 8
ROWS_PER_CORE = N_S // N_CORES          # 512
IN_ROWS = ROWS_PER_CORE + 2             # 514 (with halos)
P = 128
TILE_STARTS = [0, 126, 252, 378]        # full tiles; outputs local rows 1..504
STRIP_START = 504                       # strip rows 504..513 -> outputs 505..512
C_T = (N_T - 1) / 2.0 / T_MAT           # 2047.5

# weight layout (fp16): 4 tri blocks, identity, folded strip tri
W_TRI0 = 0                              # cols 128*t .. for t=0..3
W_IDENT = 512                           # cols 512..640: eye(128)
W_STRIP = 640                           # cols 640..660: [20,20] block-tridiag
W_COLS = 768

# main groups: output cols c0..c0+w-1, w=1024 except last (1022)
GROUP_C0 = [1, 1025, 2049, 3073]
GROUP_W = [1024, 1024, 1024, 1022]
# strip folded as 2 column-chunks x 10 rows: 4094 = 2*2047 exactly.
# chunk j covers outputs f=1..2047 <-> global cols 2047*j + f
STRIP_W = 2047
# groups whose sum-of-squares runs on DVE bn_stats instead of ACT
BN_GROUPS = [(0, 2), (1, 2), (2, 2), (3, 2), (1, 0)]
N_GROUPS = 18                           # stats cols: u = 4t+g main, 16/17 strip
BN_COLS = 12 * len(BN_GROUPS)

F32 = mybir.dt.float32
F16 = mybir.dt.float16
SUB = mybir.AluOpType.subtract
SQUARE = mybir.ActivationFunctionType.Square


def _solve_cubic(Q: float) -> float:
    c = -Q
    for _ in range(5):
        f = c ** 3 / 6.0 + c + Q
        df = 0.5 * c * c + 1.0
        c = c - f / df
    return c


C1 = _solve_cubic((B_STR - 0.0) / ALPHA)
C2 = _solve_cubic((B_STR - 1.0) / ALPHA)


def _stencil_coeffs(S: np.ndarray):
    """Per-row stencil coefficients / C_T (c folded out; re-applied via host mask)."""
    S = S.astype(np.float64)
    dS = 1.0 / (N_S - 1)
    L = C2 * S + C1 * (1.0 - S)
    dL = C2 - C1
    S_u = ALPHA * dL * (0.5 * L ** 2 + 1.0)
    S_uu = ALPHA * dL ** 2 * L
    e = 0.5 * SIGMA ** 2 * S ** 2
    f = R * S
    a_uu = e / S_u ** 2
    a_u = f / S_u - e * S_uu / S_u ** 3
    hi = a_uu / dS ** 2 + a_u / (2 * dS)
    lo = a_uu / dS ** 2 - a_u / (2 * dS)
    mid = -2.0 * a_uu / dS ** 2 - R
    return lo / C_T, mid / C_T, hi / C_T


_PROGRAM = None


def _patch_tail(tc_cls):
    """Cheaper kernel tail: drain + single barrier, no per-sem HW clears.
    Semaphore bookkeeping (free/poison) is kept so scheduling stays valid."""
    from concourse.vector_clock import ScopedClock as _SC

    def _drain_and_barrier(self, tick_clock, wait_clock):
        drain_inst = self.nc.sync.drain()
        wait_clock.add_sem_waits(drain_inst.ins, _SC({None: tick_clock.global_clock}))
        self.nc.all_engine_barrier()
        popped = self.nc._tile_sem_poison_stack.pop()
        assert popped is self._sem_poison
        sems = list(self.sems.allocated().values())
        sem_nums = [s.num if hasattr(s, "num") else s for s in sems]
        self.nc._state.prepend_free_semaphores(sem_nums)
        for poison_set in self.nc._tile_sem_poison_stack:
            poison_set.update(sem_nums)

    tc_cls._drain_and_barrier = _drain_and_barrier


def _build_program():
    if os.environ.get("BSLOSS_FAST_TAIL", "1") == "1":
        _patch_tail(tile.TileContext)
    nc = bacc.Bacc("TRN2", target_bir_lowering=False)

    v_in = nc.dram_tensor("v_in", [IN_ROWS, N_T], F16, kind="ExternalInput")
    w_in = nc.dram_tensor("w_in", [P, W_COLS], F16, kind="ExternalInput")
    stats_out = nc.dram_tensor("stats_out", [P, N_GROUPS], F32, kind="ExternalOutput")
    bn_out = nc.dram_tensor("bn_out", [P, BN_COLS], F32, kind="ExternalOutput")

    with tile.TileContext(nc) as tc:
        with (
            tc.tile_pool(name="vpool", bufs=1) as vpool,
            tc.tile_pool(name="wpool", bufs=1) as wpool,
            tc.tile_pool(name="dpool", bufs=8) as dpool,
            tc.tile_pool(name="sqpool", bufs=1) as sqpool,
            tc.tile_pool(name="psum", bufs=4, space="PSUM") as psum_pool,
        ):
            wall = wpool.tile([P, W_COLS], F16)
            nc.sync.dma_start(wall[:], w_in[:])
            stats = wpool.tile([P, N_GROUPS], F32)
            bn = wpool.tile([P, BN_COLS], F32)

            # ---- DMAs: strip chunks first (tiny), then the 4 full tiles ----
            # strip fold: chunk j reads input cols 2047j .. 2047j+2048 (2049
            # cols) into partitions 10j..10j+9 -- both chunks fully valid.
            vs = vpool.tile([20, STRIP_W + 2], F16, tag="vs")
            for j in range(2):
                c0 = STRIP_W * j
                nc.sync.dma_start(vs[10 * j:10 * j + 10, 0:STRIP_W + 2],
                                  v_in[STRIP_START:STRIP_START + 10, c0:c0 + STRIP_W + 2])
            vt = {}
            for t in range(4):
                tv = vpool.tile([P, N_T], F16, tag=f"v{t}")
                r0 = TILE_STARTS[t]
                for h in (0, 1):
                    nc.sync.dma_start(tv[:, 2048 * h:2048 * (h + 1)],
                                      v_in[r0:r0 + P, 2048 * h:2048 * (h + 1)])
                vt[t] = tv

            ident = wall[0:P, W_IDENT:W_IDENT + P]

            # ---- phase 1: all t-diff subs on DVE (fp16 2x), in DMA order ----
            ds = dpool.tile([20, STRIP_W + 1], F16, tag="ds")
            nc.vector.tensor_tensor(out=ds[0:20, 0:STRIP_W],
                                    in0=vs[0:20, 2:STRIP_W + 2],
                                    in1=vs[0:20, 0:STRIP_W], op=SUB)
            dmain = {}
            for t in range(4):
                for g in range(4):
                    c0, w = GROUP_C0[g], GROUP_W[g]
                    d = dpool.tile([P, 1024], F16, tag="d")
                    nc.vector.tensor_tensor(out=d[:, 0:w],
                                            in0=vt[t][:, c0 + 1:c0 + 1 + w],
                                            in1=vt[t][:, c0 - 1:c0 - 1 + w],
                                            op=SUB)
                    dmain[(t, g)] = d

            # ---- phase 2: PE matmuls + squares, pipelined group by group ----
            # strip first (small, fills the pipe while tiles DMA); two parts:
            # part a covers outputs f = 1+1024a .. (1024, then 1023 cols)
            stri = wall[0:20, W_STRIP:W_STRIP + 20]
            for a in (0, 1):
                f0 = 1 + 1024 * a
                aw = 1024 if a == 0 else STRIP_W - 1024
                ps = psum_pool.tile([P, 1024], F32, tag="ps")
                for ci in (0, 1):
                    cw = min(512, aw - 512 * ci)
                    nc.tensor.matmul(ps[0:20, 512 * ci:512 * ci + cw],
                                     lhsT=stri,
                                     rhs=vs[0:20, f0 + 512 * ci:f0 + 512 * ci + cw],
                                     start=True, stop=False)
                for ci in (0, 1):
                    cw = min(512, aw - 512 * ci)
                    nc.tensor.matmul(ps[0:20, 512 * ci:512 * ci + cw],
                                     lhsT=ident[0:20, 0:20],
                                     rhs=ds[0:20, f0 - 1 + 512 * ci:f0 - 1 + 512 * ci + cw],
                                     start=False, stop=True)
                sq = sqpool.tile([P, 1024], F32, tag="sq")
                nc.scalar.activation(sq[0:20, 0:aw], ps[0:20, 0:aw],
                                     SQUARE, accum_out=stats[0:20, 16 + a:17 + a])

            bn_off = {}
            for bi, tg in enumerate(BN_GROUPS):
                bn_off[tg] = 12 * bi

            for t in range(4):
                tri = wall[0:P, 128 * t:128 * (t + 1)]
                for g in range(4):
                    c0, w = GROUP_C0[g], GROUP_W[g]
                    u = 4 * t + g
                    d = dmain[(t, g)]
                    ps = psum_pool.tile([P, 1024], F32, tag="ps")
                    for ci in (0, 1):
                        cw = min(512, w - 512 * ci)
                        nc.tensor.matmul(ps[:, 512 * ci:512 * ci + cw], lhsT=tri,
                                         rhs=vt[t][:, c0 + 512 * ci:c0 + 512 * ci + cw],
                                         start=True, stop=False)
                    for ci in (0, 1):
                        cw = min(512, w - 512 * ci)
                        nc.tensor.matmul(ps[:, 512 * ci:512 * ci + cw], lhsT=ident,
                                         rhs=d[:, 512 * ci:512 * ci + cw],
                                         start=False, stop=True)
                    if (t, g) in bn_off:
                        off = bn_off[(t, g)]
                        for ci in (0, 1):
                            nc.vector.bn_stats(bn[:, off + 6 * ci:off + 6 * ci + 6],
                                               ps[:, 512 * ci:512 * (ci + 1)])
                    else:
                        sq = sqpool.tile([P, 1024], F32, tag="sq")
                        nc.scalar.activation(sq[:, 0:w], ps[:, 0:w], SQUARE,
                                             accum_out=stats[:, u:u + 1])

            nc.sync.dma_start(stats_out[:], stats[:])
            nc.sync.dma_start(bn_out[:], bn[:])

    nc.compile()
    return nc


def _host_inputs_and_masks(V: np.ndarray, S: np.ndarray):
    lo, mid, hi = _stencil_coeffs(S)
    c2 = float(C_T) ** 2

    in_maps = []
    masks = []

    for c in range(N_CORES):
        rows = np.clip(np.arange(512 * c - 1, 512 * c + 513), 0, N_S - 1)
        v_shard = V[rows, :].astype(np.float16)

        w64 = np.zeros((P, W_COLS), np.float64)
        w64[:, W_IDENT:W_IDENT + P] = np.eye(P)
        mask = np.zeros((P, N_GROUPS), np.float32)
        for t in range(4):
            t0 = TILE_STARTS[t]
            for m in range(1, 127):
                g = 512 * c - 1 + t0 + m
                if not (1 <= g <= N_S - 2):
                    continue
                w64[m - 1, 128 * t + m] = lo[g]
                w64[m, 128 * t + m] = mid[g]
                w64[m + 1, 128 * t + m] = hi[g]
                mask[m, 4 * t:4 * t + 4] = c2
        for r in range(1, 9):
            g = 512 * c - 1 + STRIP_START + r
            if not (1 <= g <= N_S - 2):
                continue
            for j in range(2):
                w64[10 * j + r - 1, W_STRIP + 10 * j + r] = lo[g]
                w64[10 * j + r, W_STRIP + 10 * j + r] = mid[g]
                w64[10 * j + r + 1, W_STRIP + 10 * j + r] = hi[g]
                mask[10 * j + r, 16] = c2
                mask[10 * j + r, 17] = c2
        in_maps.append({"v_in": v_shard, "w_in": w64.astype(np.float16)})
        masks.append(mask)
    return in_maps, masks


_LAST_RESULTS = None  # stashed BassKernelResults (for the test harness)


def kernel(V_norm: np.ndarray, S_grid: np.ndarray, t_grid: np.ndarray):
    global _PROGRAM, _LAST_RESULTS

    V = np.asarray(V_norm, dtype=np.float32).reshape(N_S, N_T)
    S = np.asarray(S_grid, dtype=np.float32).reshape(N_S)
    t = np.asarray(t_grid, dtype=np.float32).reshape(N_T)

    if _PROGRAM is None:
        _PROGRAM = _build_program()
    nc = _PROGRAM

    in_maps, masks = _host_inputs_and_masks(V, S)
    trace = bool(os.environ.get("BSLOSS_TRACE"))
    res = run_bass_kernel_spmd(nc, in_maps, core_ids=list(range(N_CORES)),
                               trace=trace)
    _LAST_RESULTS = res

    pde_sum = 0.0
    for c in range(N_CORES):
        stats = res.results[c]["stats_out"].astype(np.float64)
        bn = res.results[c]["bn_out"].astype(np.float64)
        per_part = stats
        for bi, (bt, bg) in enumerate(BN_GROUPS):
            u = 4 * bt + bg
            rec = bn[:, 12 * bi:12 * bi + 12].reshape(P, 2, 2, 3)
            n_, mean_, m2_ = rec[..., 0], rec[..., 1], rec[..., 2]
            per_part[:, u] = (m2_ + n_ * mean_ * mean_).sum(axis=(1, 2))
        m = masks[c].astype(np.float64)
        pde_sum += float(np.where(m > 0, per_part * m, 0.0).sum())
    n_int = (N_S - 2) * (N_T - 2)
    pde_loss = pde_sum / n_int

    # ---- boundary losses on host (tiny O(N) edge terms), float64 ----
    V64 = V.astype(np.float64)
    S64 = S.astype(np.float64)
    t64 = t.astype(np.float64)

    loss_S0 = float((V64[0, :] ** 2).sum() / N_T)

    tau = 1.0 - t64
    V_ff = 1.0 - K * np.exp(-R * tau) / SMAX
    loss_Smax = float(((V64[N_S - 1, :] - V_ff) ** 2).sum() / N_T)

    x = SOFTPLUS_BETA * (S64 - K / SMAX)
    payoff = (np.maximum(x, 0.0) + np.log1p(np.exp(-np.abs(x)))) / SOFTPLUS_BETA
    diff_T = V64[:, N_T - 1] - payoff
    abs_d = np.abs(diff_T)
    huber = np.where(abs_d < HUBER_DELTA, 0.5 * diff_T ** 2,
                     HUBER_DELTA * (abs_d - 0.5 * HUBER_DELTA))
    loss_T = float(huber.sum() / N_S)

    total = L_PDE * pde_loss + L_BC * loss_Smax + L_TC * loss_T
    return (np.float32(total), np.float32(pde_loss), np.float32(loss_S0),
            np.float32(loss_Smax), np.float32(loss_T))
